# revision 1
# baseline (speedup 1.0000x reference)
"""KANvolution Trainium2 Bass kernel.

Math: the reference evaluates, per patch element x and per (f,c,ki,kj):
    K(x) = w_spline * sum_g basis_g(clip(x)) * cp_g  +  w_silu * silu(x)
with basis = normalized linear B-spline hats on a uniform 17-knot grid in
[-1,1].  The hat interpolant is piecewise-linear, so it can be rewritten
exactly as
    spline(x) = v0 + sum_{k=0..15} coef_k * relu(clip(x) - g_k)
(v0 folds into the bias).  That turns the whole module into a standard
3x3 valid conv over 17 feature maps of x (16 relus + silu), i.e. 9 taps
of matmuls with contraction K = 17*32 = 544 (padded to 5 k-tiles of 128,
with a constant-1 row carrying the bias).

Sharding: 8 cores = (batch b, output-row half).  Each core gets a
(34, 66, 32) input slab (2-row halo) and produces (32, 64, 64).

Device pipeline per core:
  DMA x (natural [spatial,c]) -> 18 PE transposes -> x^T [32c, 2304s]
  -> replicate x4 along partitions (SBUF->SBUF DMA) -> clip (DVE) ->
  4 relu k-tiles + silu k-tile (ACT, per-partition bias = -g) ->
  4 output chunks x 45 accumulating matmuls (float32r, N=512) ->
  PSUM -> copy -> PE transpose -> [128 m, 64 f] -> DMA out.
"""

import numpy as np
from contextlib import ExitStack

import concourse.bacc as bacc
import concourse.mybir as mybir
import concourse.tile as tile
from concourse.bass_utils import run_bass_kernel_spmd

# Problem constants (hardcoded per harness contract)
B, H, W, C, F = 4, 66, 66, 32, 64
KH = KW = 3
G = 16                      # spline intervals; G+1 = 17 knots
GRID_H = 2.0 / G            # 0.125
HO, WO = H - KH + 1, W - KW + 1          # 64, 64
N_CORES = 8
ROWS_PER_CORE = HO // 2                  # 32 output rows
IN_ROWS = ROWS_PER_CORE + KH - 1         # 34 input rows
SPAT = IN_ROWS * W                       # 2244 input spatial positions
SPAT_PAD = 18 * 128                      # 2304 (pad to 18 tiles of 128)
N_TAPS = KH * KW                         # 9
N_KTILES = 5                             # ceil(17*32/128) -> 4 relu tiles + silu tile
CHUNK_ROWS = 8                           # output rows per matmul chunk
N_CHUNKS = ROWS_PER_CORE // CHUNK_ROWS   # 4
NFREE = CHUNK_ROWS * WO                  # 512 moving-dim per matmul

_COMPILED = None  # cached (nc) program
import os
MM_DT = os.environ.get("KAN_MM_DT", "float32r")  # matmul operand dtype knob



def _build_weights(control_points, w_spline, w_silu, bias):
    """Host-side transform of the KAN params into conv-as-matmul weights.

    Returns w_host [128, 45*64] float32 and nothing else (bias folded in).
    Layout: w_host[p, (tap*5 + t)*64 + f] = W[tap][t][p, f] where row
    p = r*32 + c of k-tile t encodes feature g = 4t + r (t<4), and k-tile 4
    holds silu on rows 0..31, the bias row at partition 32 (tap 0 only).
    """
    cp = control_points.astype(np.float64)
    ws = w_spline.astype(np.float64)
    # hat interpolant values at the knots; the reference divides the hat
    # weights by (sum + 1e-8) with sum == 1, i.e. a uniform 1/(1+1e-8) scale
    v = ws[..., None] * cp / (1.0 + 1e-8)          # (F, C, 3, 3, 17)
    s = np.diff(v, axis=-1) / GRID_H               # (F, C, 3, 3, 16) slopes
    coef = s.copy()
    coef[..., 1:] = s[..., 1:] - s[..., :-1]       # slope deltas at knots 1..15
    v0 = v[..., 0]                                 # value at x = -1
    bias_eff = bias.astype(np.float64) + v0.sum(axis=(1, 2, 3))   # (F,)

    w_all = np.zeros((N_TAPS, N_KTILES, 128, F), dtype=np.float64)
    for i in range(KH):
        for j in range(KW):
            tap = i * KW + j
            for t in range(4):
                for r in range(4):
                    g = 4 * t + r
                    # rows r*32..r*32+32 = channels, feature g
                    w_all[tap, t, r * 32:(r + 1) * 32, :] = coef[:, :, i, j, g].T
            w_all[tap, 4, 0:32, :] = w_silu[:, :, i, j].astype(np.float64).T
    w_all[0, 4, 32, :] = bias_eff                  # constant-1 row, tap 0 only
    w_host = w_all.transpose(2, 0, 1, 3).reshape(128, N_TAPS * N_KTILES * F)
    import ml_dtypes
    npdt = np.float32 if MM_DT.startswith("float32") else ml_dtypes.bfloat16
    return np.ascontiguousarray(w_host.astype(npdt))


def _build_program():
    nc = bacc.Bacc("TRN2", target_bir_lowering=False, debug=False,
                   num_devices=N_CORES)
    f32 = mybir.dt.float32
    f32r = getattr(mybir.dt, MM_DT)

    xs_in = nc.declare_dram_parameter("xs", [SPAT_PAD, C], f32, isOutput=False)
    w_in = nc.declare_dram_parameter("w", [128, N_TAPS * N_KTILES * F], f32r,
                                     isOutput=False)
    gb_in = nc.declare_dram_parameter("gb", [128, 4], f32, isOutput=False)
    id_in = nc.declare_dram_parameter("ident", [128, 128], f32, isOutput=False)
    y_out = nc.declare_dram_parameter("y", [ROWS_PER_CORE * WO, F], f32,
                                      isOutput=True)

    with tile.TileContext(nc) as tc:
        with ExitStack() as ctx:
            sb = ctx.enter_context(tc.tile_pool(name="sb", bufs=1))
            ps_x = ctx.enter_context(tc.tile_pool(name="psx", bufs=2, space="PSUM"))
            ps_o = ctx.enter_context(tc.tile_pool(name="pso", bufs=3, space="PSUM"))
            ps_t = ctx.enter_context(tc.tile_pool(name="pst", bufs=2, space="PSUM"))
            ob = ctx.enter_context(tc.tile_pool(name="ob", bufs=2))

            # --- static inputs ---
            w_sb = sb.tile([128, N_TAPS * N_KTILES * F], f32r, tag="w")
            nc.sync.dma_start(w_sb[:], w_in[:])
            gb_sb = sb.tile([128, 4], f32, tag="gb")
            nc.sync.dma_start(gb_sb[:], gb_in[:])
            id_sb = sb.tile([128, 128], f32, tag="id")
            nc.sync.dma_start(id_sb[:], id_in[:])

            # --- load x natural layout: x_nat[p, (t,c)] = xs[t*128+p, c] ---
            x_nat = sb.tile([128, 18 * C], f32, tag="xnat")
            nc.sync.dma_start(
                x_nat[:].rearrange("p (t c) -> p t c", c=C),
                xs_in[:].rearrange("(t p) c -> p t c", p=128),
            )

            # --- transpose to x_rep[0:32] = x^T [c, spatial] ---
            x_rep = sb.tile([128, SPAT_PAD], f32, tag="xrep")
            for g in range(5):                       # groups of 4 transposes
                n_t = 4 if g < 4 else 2
                ps = ps_x.tile([32, 128 * n_t], f32, tag="psx")
                for u in range(n_t):
                    t = 4 * g + u
                    nc.tensor.transpose(
                        ps[:, 128 * u:128 * (u + 1)],
                        x_nat[:, C * t:C * (t + 1)],
                        id_sb[:],
                    )
                nc.scalar.copy(x_rep[0:32, 512 * g:512 * g + 128 * n_t], ps[:])

            # --- features, in 2 spatial halves so chunk-0 matmuls start early ---
            HALF = SPAT_PAD // 2
            xc = sb.tile([128, SPAT_PAD], f32, tag="xc")
            feats = [sb.tile([128, SPAT_PAD], f32r, name=f"feat{t}",
                             tag=f"feat{t}") for t in range(4)]
            f_silu = sb.tile([128, SPAT_PAD], f32r, tag="feat4")
            feats.append(f_silu)

            def make_features(h):
                cs = slice(HALF * h, HALF * (h + 1))
                # replicate x4 along partitions (SBUF->SBUF DMA)
                nc.sync.dma_start(x_rep[32:64, cs], x_rep[0:32, cs])
                nc.sync.dma_start(x_rep[64:96, cs], x_rep[0:32, cs])
                nc.sync.dma_start(x_rep[96:128, cs], x_rep[0:32, cs])
                nc.vector.tensor_scalar(xc[:, cs], x_rep[:, cs], 1.0, -1.0,
                                        mybir.AluOpType.min, mybir.AluOpType.max)
                for t in range(4):
                    if t < 2:   # ACT: relu(x + (-g))
                        nc.scalar.activation(feats[t][:, cs], xc[:, cs],
                                             mybir.ActivationFunctionType.Relu,
                                             bias=gb_sb[:, t:t + 1], scale=1.0)
                    else:       # DVE: (x - g) max 0, per-partition g
                        nc.vector.tensor_scalar(feats[t][:, cs], xc[:, cs],
                                                gb_sb[:, t:t + 1], 0.0,
                                                mybir.AluOpType.add,
                                                mybir.AluOpType.max)
                nc.scalar.activation(f_silu[:, cs], x_rep[:, cs],
                                     mybir.ActivationFunctionType.Silu)
                # constant-1 bias row: (x*0)+1 on DVE (memset can't write f32r)
                nc.vector.tensor_scalar(f_silu[32:33, cs], xc[32:33, cs],
                                        0.0, 1.0,
                                        mybir.AluOpType.mult,
                                        mybir.AluOpType.add)

            make_features(0)

            # --- conv as accumulating matmuls ---
            for q in range(N_CHUNKS):
                if q == 1:
                    make_features(1)
                po = ps_o.tile([F, NFREE], f32, tag="po")
                first = True
                for i in range(KH):
                    for j in range(KW):
                        tap = i * KW + j
                        base = (CHUNK_ROWS * q + i) * W
                        for t in range(N_KTILES):
                            rhs = (feats[t][:, base:base + CHUNK_ROWS * W]
                                   .rearrange("p (r w) -> p r w", w=W)
                                   [:, :, j:j + WO])
                            last = (tap == N_TAPS - 1) and (t == N_KTILES - 1)
                            col = (tap * N_KTILES + t) * F
                            nc.tensor.matmul(
                                po[:].rearrange("f (r w) -> f r w", w=WO),
                                w_sb[:, col:col + F],
                                rhs,
                                start=first, stop=last,
                            )
                            first = False

                # psum [64f, 512m] -> sbuf -> transpose -> [128m, 64f] -> DMA
                o_sb = ob.tile([F, NFREE], f32, tag="osb")
                nc.scalar.copy(o_sb[:], po[:])
                stage = ob.tile([128, 4 * F], f32, tag="stage")
                for u in range(4):
                    pt = ps_t.tile([128, F], f32, tag="pt")
                    nc.tensor.transpose(pt[:], o_sb[:, 128 * u:128 * (u + 1)],
                                        id_sb[0:F, 0:F])
                    nc.vector.tensor_copy(stage[:, F * u:F * (u + 1)], pt[:])
                nc.sync.dma_start(
                    y_out[NFREE * q:NFREE * (q + 1), :]
                        .rearrange("(tb p) f -> p tb f", p=128),
                    stage[:].rearrange("p (tb f) -> p tb f", f=F),
                )

    nc.compile()
    return nc


def _get_program():
    global _COMPILED
    if _COMPILED is None:
        _COMPILED = _build_program()
    return _COMPILED


def kernel(x, control_points, w_spline, w_silu, bias):
    x = np.asarray(x, dtype=np.float32)
    control_points = np.asarray(control_points, dtype=np.float32)
    w_spline = np.asarray(w_spline, dtype=np.float32)
    w_silu = np.asarray(w_silu, dtype=np.float32)
    bias = np.asarray(bias, dtype=np.float32)

    w_host = _build_weights(control_points, w_spline, w_silu, bias)
    grid = np.linspace(-1.0, 1.0, G + 1, dtype=np.float64)
    gb = np.zeros((128, 4), dtype=np.float32)
    for t in range(4):
        for p in range(128):
            gb[p, t] = -grid[4 * t + p // 32]
    ident = np.eye(128, dtype=np.float32)

    in_maps = []
    for core in range(N_CORES):
        b, half = divmod(core, 2)
        r0 = half * ROWS_PER_CORE
        xs = np.zeros((SPAT_PAD, C), dtype=np.float32)
        xs[:SPAT] = x[b, r0:r0 + IN_ROWS].reshape(SPAT, C)
        in_maps.append({"xs": xs, "w": w_host, "gb": gb, "ident": ident})

    nc = _get_program()
    res = run_bass_kernel_spmd(nc, in_maps, list(range(N_CORES)))

    out = np.empty((B, HO, WO, F), dtype=np.float32)
    for core in range(N_CORES):
        b, half = divmod(core, 2)
        r0 = half * ROWS_PER_CORE
        out[b, r0:r0 + ROWS_PER_CORE] = res.results[core]["y"].reshape(
            ROWS_PER_CORE, WO, F)
    return out



# revision 3
# speedup vs baseline: 1.6904x; 1.6904x over previous
"""KANvolution Trainium2 Bass kernel.

Math: the reference evaluates, per patch element x and per (f,c,ki,kj):
    K(x) = w_spline * sum_g basis_g(clip(x)) * cp_g  +  w_silu * silu(x)
with basis = normalized linear B-spline hats on a uniform 17-knot grid in
[-1,1].  The hat interpolant is piecewise-linear, so it is rewritten exactly as
    spline(x) = v0 + sum_{k=0..15} coef_k * relu(min(x,1) - g_k)
(v0 folds into the bias; the lower clip is redundant under the relu).  That
turns the whole module into a standard 3x3 valid conv over 17 feature maps
of x (16 relus + silu), contraction K = 9 taps x 544 (+1 bias row).

Sharding: 8 cores = (batch b, output-row half).  Each core gets a
(34, 66, 32) input slab (2-row halo) and produces (32, 64, 64).

Device pipeline per core (v2):
  - host pre-transposes x to [32c, spatial] (fast 32-descriptor DMA, no PE
    transposes) and post-transposes the [64f, spatial] output back.
  - features: replicate x to 128 partitions (4 g-groups x 32c), one DVE min,
    4 relu tiles (ACT/DVE), silu; the 9 silu k-tiles are packed into 3 by
    materializing tap-shifted silu copies on partition groups (plus a
    constant-1 bias row), K=33 for the last pack.  All bf16.
  - matmuls: 39 k-tiles x 4 output chunks.  Chunks are processed in
    concurrent PAIRS on the two 64-column halves of the PE array
    (tile_position col groups, weights duplicated to both halves), so each
    ~213ns matmul slot retires two k-tile matmuls.  fp32 PSUM, no adds.
  - ~3.4us of tiny warmup matmuls at t=0 keep the HAM clock-gate warm so the
    real stream runs at 2.4 GHz from the start.
"""

import numpy as np
from contextlib import ExitStack

import ml_dtypes
import concourse.bacc as bacc
import concourse.mybir as mybir
import concourse.tile as tile
from concourse.bass_utils import run_bass_kernel_spmd

# Problem constants (hardcoded per harness contract)
B, H, W, C, F = 4, 66, 66, 32, 64
KH = KW = 3
G = 16                      # spline intervals; G+1 = 17 knots
GRID_H = 2.0 / G            # 0.125
HO, WO = H - KH + 1, W - KW + 1          # 64, 64
N_CORES = 8
ROWS_PER_CORE = HO // 2                  # 32 output rows
IN_ROWS = ROWS_PER_CORE + KH - 1         # 34 input rows
SPAT = IN_ROWS * W                       # 2244 input spatial positions
SPAT_PAD = 18 * 128                      # 2304 (padded)
N_TAPS = KH * KW                         # 9
N_KT = 37 * 128 + 33                     # contraction rows (36 relu kt + packs)
N_KTILES = 39                            # 36 relu + 3 silu packs
CHUNK_ROWS = 8                           # output rows per matmul chunk
N_CHUNKS = ROWS_PER_CORE // CHUNK_ROWS   # 4
NFREE = CHUNK_ROWS * WO                  # 512 moving-dim per matmul
PACK_SHIFTS = [[0, 1, 2, W], [W + 1, W + 2, 2 * W, 2 * W + 1], [2 * W + 2]]
MAX_SHIFT = 2 * W + 2                    # 134
# feature col ranges per half: chunk-pair 0 (chunks 0,1) reads cols < 1188
FEAT_SPLIT = 1216
SILU_SPLIT = FEAT_SPLIT + MAX_SHIFT      # 1350
PACK_END = SPAT_PAD - MAX_SHIFT          # 2170 (cols beyond never read)
WARMUP_MMS = 52

_COMPILED = None


def _build_weights(control_points, w_spline, w_silu, bias):
    """Host-side transform of the KAN params into conv-as-matmul weights.

    Returns w_host [128, 39*128] bf16.  k-tile kt occupies cols
    kt*128..kt*128+128, duplicated as [Wkt | Wkt] so the two PE column
    halves can each hold a copy.  kt = tap*4+t (t-th relu g-group, row
    r*32+c encodes g=4t+r); kt 36/37 = silu packs A/B (row 32r+c = tap
    group r); kt 38 = silu tap 8 (rows 0..31) + bias row 32.
    """
    cp = control_points.astype(np.float64)
    ws = w_spline.astype(np.float64)
    # hat interpolant values at the knots; the reference divides the hat
    # weights by (sum + 1e-8) with sum == 1, i.e. a uniform scale
    v = ws[..., None] * cp / (1.0 + 1e-8)          # (F, C, 3, 3, 17)
    s = np.diff(v, axis=-1) / GRID_H               # (F, C, 3, 3, 16) slopes
    coef = s.copy()
    coef[..., 1:] = s[..., 1:] - s[..., :-1]       # slope deltas at knots 1..15
    v0 = v[..., 0]                                 # value at x = -1
    bias_eff = bias.astype(np.float64) + v0.sum(axis=(1, 2, 3))   # (F,)
    wsilu = w_silu.astype(np.float64)

    w_all = np.zeros((N_KTILES, 128, F), dtype=np.float64)
    for i in range(KH):
        for j in range(KW):
            tap = i * KW + j
            for t in range(4):
                for r in range(4):
                    g = 4 * t + r
                    w_all[tap * 4 + t, r * 32:(r + 1) * 32, :] = coef[:, :, i, j, g].T
    for pk, taps in enumerate([[0, 1, 2, 3], [4, 5, 6, 7], [8]]):
        for r, tap in enumerate(taps):
            i, j = divmod(tap, KW)
            w_all[36 + pk, r * 32:(r + 1) * 32, :] = wsilu[:, :, i, j].T
    w_all[38, 32, :] = bias_eff                    # constant-1 row
    w_host = np.concatenate([w_all, w_all], axis=2)     # [39, 128, 128]
    w_host = w_host.transpose(1, 0, 2).reshape(128, N_KTILES * 2 * F)
    return np.ascontiguousarray(w_host.astype(ml_dtypes.bfloat16))


def _build_program():
    nc = bacc.Bacc("TRN2", target_bir_lowering=False, debug=False,
                   num_devices=N_CORES)
    f32 = mybir.dt.float32
    bf16 = mybir.dt.bfloat16

    xs_in = nc.declare_dram_parameter("xs", [C, SPAT_PAD], bf16, isOutput=False)
    w_in = nc.declare_dram_parameter("w", [128, N_KTILES * 2 * F], bf16,
                                     isOutput=False)
    gb_in = nc.declare_dram_parameter("gb", [128, 4], f32, isOutput=False)
    y_out = nc.declare_dram_parameter("y", [F, N_CHUNKS * NFREE], f32,
                                      isOutput=True)

    with tile.TileContext(nc) as tc:
        with ExitStack() as ctx:
            sb = ctx.enter_context(tc.tile_pool(name="sb", bufs=1))
            ps = ctx.enter_context(tc.tile_pool(name="ps", bufs=2, space="PSUM"))
            ps_w = ctx.enter_context(tc.tile_pool(name="psw", bufs=1, space="PSUM"))
            ob = ctx.enter_context(tc.tile_pool(name="ob", bufs=2))

            # --- static inputs ---
            gb_sb = sb.tile([128, 4], f32, tag="gb")
            nc.sync.dma_start(gb_sb[:], gb_in[:])
            w_sb = sb.tile([128, N_KTILES * 2 * F], bf16, tag="w")
            half_w = 20 * 2 * F
            nc.sync.dma_start(w_sb[:, :half_w], w_in[:, :half_w])
            nc.sync.dma_start(w_sb[:, half_w:], w_in[:, half_w:])

            # --- PE warmup: tiny matmuls keep HAM at K=8/8 while features
            # are being built (gb is ready almost immediately) ---
            pw = ps_w.tile([4, 4], f32, tag="pw")
            for _ in range(WARMUP_MMS):
                nc.tensor.matmul(pw[:], gb_sb[:, 0:4], gb_sb[:], start=True,
                                 stop=True)

            # --- x: [32c, spat] then replicate to 4 partition groups ---
            x_rep = sb.tile([128, SPAT_PAD], bf16, tag="xrep")
            nc.sync.dma_start(x_rep[0:32, :], xs_in[:])
            for r in range(1, 4):
                nc.sync.dma_start(x_rep[32 * r:32 * (r + 1), :], x_rep[0:32, :])

            xc = sb.tile([128, SPAT_PAD], bf16, tag="xc")
            feats = [sb.tile([128, SPAT_PAD], bf16, name=f"feat{t}",
                             tag=f"feat{t}") for t in range(4)]
            silu_rep = sb.tile([128, SPAT_PAD], bf16, tag="silu")
            packs = [sb.tile([128, SPAT_PAD], bf16, name=f"pack{p}",
                             tag=f"pack{p}") for p in range(3)]

            def make_features(h):
                fs = slice(FEAT_SPLIT * h, FEAT_SPLIT if h == 0 else SPAT_PAD)
                ss = slice(SILU_SPLIT * h, SILU_SPLIT if h == 0 else SPAT_PAD)
                pc = slice(FEAT_SPLIT * h, FEAT_SPLIT if h == 0 else PACK_END)
                # silu first so the pack DMAs can start early
                nc.scalar.activation(silu_rep[:, ss], x_rep[:, ss],
                                     mybir.ActivationFunctionType.Silu)
                nc.vector.tensor_scalar_min(xc[:, fs], x_rep[:, fs], 1.0)
                for t in range(4):
                    if t < 2:   # ACT: relu(x + (-g))
                        nc.scalar.activation(feats[t][:, fs], xc[:, fs],
                                             mybir.ActivationFunctionType.Relu,
                                             bias=gb_sb[:, t:t + 1], scale=1.0)
                    else:       # DVE: (x + (-g)) max 0, per-partition g
                        nc.vector.tensor_scalar(feats[t][:, fs], xc[:, fs],
                                                gb_sb[:, t:t + 1], 0.0,
                                                mybir.AluOpType.add,
                                                mybir.AluOpType.max)
                # tap-shifted silu copies on partition groups
                a, b = pc.start, pc.stop
                for p in range(3):
                    for r, sh in enumerate(PACK_SHIFTS[p]):
                        rows = slice(32 * r, 32 * (r + 1))
                        nc.sync.dma_start(packs[p][rows, a:b],
                                          silu_rep[rows, a + sh:b + sh])
                # constant-1 bias row (reads real silu values: x*0+1, no NaNs)
                nc.vector.tensor_scalar(packs[2][32:33, a:b],
                                        silu_rep[32:33, a:b], 0.0, 1.0,
                                        mybir.AluOpType.mult,
                                        mybir.AluOpType.add)

            make_features(0)
            make_features(1)

            # --- conv: 39 k-tiles, chunk pairs on PE column halves ---
            def rhs_for(kt, q):
                if kt < 36:
                    tap, t = divmod(kt, 4)
                    i, j = divmod(tap, KW)
                    base = (CHUNK_ROWS * q + i) * W
                    return (feats[t][:, base:base + CHUNK_ROWS * W]
                            .rearrange("p (r w) -> p r w", w=W)[:, :, j:j + WO])
                p = kt - 36
                k = 33 if p == 2 else 128
                base = CHUNK_ROWS * q * W
                return (packs[p][0:k, base:base + CHUNK_ROWS * W]
                        .rearrange("p (r w) -> p r w", w=W)[:, :, 0:WO])

            for cp in range(N_CHUNKS // 2):
                po = ps.tile([128, NFREE], f32, tag="po")
                for kt in range(N_KTILES):
                    k = 33 if kt == 38 else 128
                    col = kt * 2 * F
                    for half in range(2):
                        q = 2 * cp + half
                        nc.tensor.matmul(
                            po[F * half:F * (half + 1), :]
                            .rearrange("f (r w) -> f r w", w=WO),
                            w_sb[0:k, col + F * half:col + F * (half + 1)],
                            rhs_for(kt, q),
                            start=(kt == 0), stop=(kt == N_KTILES - 1),
                        )
                o_sb = ob.tile([128, NFREE], f32, tag="osb")
                nc.scalar.copy(o_sb[0:F, :], po[0:F, :])
                nc.vector.tensor_copy(o_sb[F:128, :], po[F:128, :])
                for half in range(2):
                    q = 2 * cp + half
                    nc.sync.dma_start(
                        y_out[:, NFREE * q:NFREE * (q + 1)],
                        o_sb[F * half:F * (half + 1), :])

    nc.compile()
    return nc


def _get_program():
    global _COMPILED
    if _COMPILED is None:
        _COMPILED = _build_program()
    return _COMPILED


def build_in_maps(x, control_points, w_spline, w_silu, bias):
    x = np.asarray(x, dtype=np.float32)
    w_host = _build_weights(np.asarray(control_points, dtype=np.float32),
                            np.asarray(w_spline, dtype=np.float32),
                            np.asarray(w_silu, dtype=np.float32),
                            np.asarray(bias, dtype=np.float32))
    grid = np.linspace(-1.0, 1.0, G + 1, dtype=np.float64)
    gb = np.zeros((128, 4), dtype=np.float32)
    for t in range(4):
        for p in range(128):
            gb[p, t] = -grid[4 * t + p // 32]

    in_maps = []
    for core in range(N_CORES):
        b, half = divmod(core, 2)
        r0 = half * ROWS_PER_CORE
        xs = np.zeros((C, SPAT_PAD), dtype=np.float32)
        xs[:, :SPAT] = x[b, r0:r0 + IN_ROWS].reshape(SPAT, C).T
        in_maps.append({"xs": xs.astype(ml_dtypes.bfloat16), "w": w_host,
                        "gb": gb})
    return in_maps


def unshard_output(results):
    out = np.empty((B, HO, WO, F), dtype=np.float32)
    for core in range(N_CORES):
        b, half = divmod(core, 2)
        r0 = half * ROWS_PER_CORE
        y = results[core]["y"]                       # [F, 2048]
        out[b, r0:r0 + ROWS_PER_CORE] = (
            y.reshape(F, ROWS_PER_CORE, WO).transpose(1, 2, 0))
    return out


def kernel(x, control_points, w_spline, w_silu, bias):
    in_maps = build_in_maps(x, control_points, w_spline, w_silu, bias)
    nc = _get_program()
    res = run_bass_kernel_spmd(nc, in_maps, list(range(N_CORES)))
    return unshard_output(res.results)


# revision 4
# speedup vs baseline: 1.8947x; 1.1209x over previous
"""KANvolution Trainium2 Bass kernel.

Math: the reference evaluates, per patch element x and per (f,c,ki,kj):
    K(x) = w_spline * sum_g basis_g(clip(x)) * cp_g  +  w_silu * silu(x)
with basis = normalized linear B-spline hats on a uniform 17-knot grid in
[-1,1].  The hat interpolant is piecewise-linear, so it is rewritten exactly as
    spline(x) = v0 + sum_{k=0..15} coef_k * relu(min(x,1) - g_k)
(v0 folds into the bias; the lower clip is redundant under the relu).  That
turns the whole module into a standard 3x3 valid conv over 17 feature maps
of x (16 relus + silu), contraction K = 9 taps x 544 (+1 bias row).

Sharding: 8 cores = (batch b, output-row half).  Each core gets a
(34, 66, 32) input slab (2-row halo) and produces (32, 64, 64).

Device pipeline per core (v3):
  - the host pre-computes every layout transform: x clipped+replicated to
    4 partition groups [128, spat] for the relu features, tap-shifted raw-x
    copies for the packed silu k-tiles (9 silu taps packed into 3 k-tiles),
    and the output is shipped back as [64f, spat] and transposed on host.
    Device-side there are only plain queue DMAs (no serialized SP DIRECT2D
    copies, no PE transposes).
  - features (bf16): 4 relu tiles on DVE (per-partition knot bias), 3
    silu tiles on ACT, emitted in 4 column ranges so the matmul stream
    starts after ~1/4 of the feature work.
  - matmuls (bf16): 39 k-tiles x 4 output chunks.  Chunks are processed in
    concurrent PAIRS on the two 64-column halves of the PE array
    (tile_position col groups via out base partition, weights duplicated to
    both halves), so each ~213ns matmul slot retires two k-tile matmuls.
  - tiny warmup matmuls + a dummy silu at t=0 pre-warm the HAM clock gate
    and the ACT function table while the input DMAs run.
"""

import numpy as np
from contextlib import ExitStack

import ml_dtypes
import concourse.bacc as bacc
import concourse.mybir as mybir
import concourse.tile as tile
from concourse.bass_utils import run_bass_kernel_spmd

# Problem constants (hardcoded per harness contract)
B, H, W, C, F = 4, 66, 66, 32, 64
KH = KW = 3
G = 16                      # spline intervals; G+1 = 17 knots
GRID_H = 2.0 / G            # 0.125
HO, WO = H - KH + 1, W - KW + 1          # 64, 64
N_CORES = 8
ROWS_PER_CORE = HO // 2                  # 32 output rows
IN_ROWS = ROWS_PER_CORE + KH - 1         # 34 input rows
SPAT = IN_ROWS * W                       # 2244 input spatial positions
SPAT_PAD = 2304                          # padded
N_TAPS = KH * KW                         # 9
N_KTILES = 39                            # 36 relu + 3 silu packs
CHUNK_ROWS = 8                           # output rows per matmul chunk
N_CHUNKS = ROWS_PER_CORE // CHUNK_ROWS   # 4
NFREE = CHUNK_ROWS * WO                  # 512 moving-dim per matmul
PACK_SHIFTS = [[0, 1, 2, W], [W + 1, W + 2, 2 * W, 2 * W + 1], [2 * W + 2]]
RANGES = [0, 608, 1216, 1760, SPAT_PAD]  # feature column ranges
WARMUP_MMS = 80

_COMPILED = None


def _build_weights(control_points, w_spline, w_silu, bias):
    """Host-side transform of the KAN params into conv-as-matmul weights.

    Returns w_host [128, 39*128] bf16.  k-tile kt occupies cols
    kt*128..kt*128+128, duplicated as [Wkt | Wkt] so the two PE column
    halves each hold a copy.  kt = tap*4+t (t-th relu g-group, row
    r*32+c encodes g=4t+r); kt 36/37 = silu packs A/B (row 32r+c = tap
    group r); kt 38 = silu tap 8 (rows 0..31) + bias row 32.
    """
    cp = control_points.astype(np.float64)
    ws = w_spline.astype(np.float64)
    # hat interpolant values at the knots; the reference divides the hat
    # weights by (sum + 1e-8) with sum == 1, i.e. a uniform scale
    v = ws[..., None] * cp / (1.0 + 1e-8)          # (F, C, 3, 3, 17)
    s = np.diff(v, axis=-1) / GRID_H               # (F, C, 3, 3, 16) slopes
    coef = s.copy()
    coef[..., 1:] = s[..., 1:] - s[..., :-1]       # slope deltas at knots 1..15
    v0 = v[..., 0]                                 # value at x = -1
    bias_eff = bias.astype(np.float64) + v0.sum(axis=(1, 2, 3))   # (F,)
    wsilu = w_silu.astype(np.float64)

    w_all = np.zeros((N_KTILES, 128, F), dtype=np.float64)
    for i in range(KH):
        for j in range(KW):
            tap = i * KW + j
            for t in range(4):
                for r in range(4):
                    g = 4 * t + r
                    w_all[tap * 4 + t, r * 32:(r + 1) * 32, :] = coef[:, :, i, j, g].T
    for pk, taps in enumerate([[0, 1, 2, 3], [4, 5, 6, 7], [8]]):
        for r, tap in enumerate(taps):
            i, j = divmod(tap, KW)
            w_all[36 + pk, r * 32:(r + 1) * 32, :] = wsilu[:, :, i, j].T
    w_all[38, 32, :] = bias_eff                    # constant-1 row
    w_host = np.concatenate([w_all, w_all], axis=2)     # [39, 128, 128]
    w_host = w_host.transpose(1, 0, 2).reshape(128, N_KTILES * 2 * F)
    return np.ascontiguousarray(w_host.astype(ml_dtypes.bfloat16))


def _build_program():
    nc = bacc.Bacc("TRN2", target_bir_lowering=False, debug=False,
                   num_devices=N_CORES)
    f32 = mybir.dt.float32
    bf16 = mybir.dt.bfloat16

    # host-prepared inputs: xr = min(x,1) replicated to 4 partition groups;
    # xa/xb/xc = raw x, tap-shifted per partition group (for packed silu)
    xr_in = nc.declare_dram_parameter("xr", [128, SPAT_PAD], bf16, isOutput=False)
    xa_in = nc.declare_dram_parameter("xa", [128, SPAT_PAD], bf16, isOutput=False)
    xb_in = nc.declare_dram_parameter("xb", [128, SPAT_PAD], bf16, isOutput=False)
    xc_in = nc.declare_dram_parameter("xc", [33, SPAT_PAD], bf16, isOutput=False)
    w_in = nc.declare_dram_parameter("w", [128, N_KTILES * 2 * F], bf16,
                                     isOutput=False)
    gb_in = nc.declare_dram_parameter("gb", [128, 4], f32, isOutput=False)
    y_out = nc.declare_dram_parameter("y", [F, N_CHUNKS * NFREE], f32,
                                      isOutput=True)

    with tile.TileContext(nc) as tc:
        with ExitStack() as ctx:
            sb = ctx.enter_context(tc.tile_pool(name="sb", bufs=1))
            ps = ctx.enter_context(tc.tile_pool(name="ps", bufs=2, space="PSUM"))
            ps_w = ctx.enter_context(tc.tile_pool(name="psw", bufs=1, space="PSUM"))
            ob = ctx.enter_context(tc.tile_pool(name="ob", bufs=2))

            # --- input DMAs (x first: features gate the matmul stream) ---
            xr = sb.tile([128, SPAT_PAD], bf16, tag="xr")
            nc.sync.dma_start(xr[:], xr_in[:])
            xa = sb.tile([128, SPAT_PAD], bf16, tag="xa")
            nc.sync.dma_start(xa[:], xa_in[:])
            xb = sb.tile([128, SPAT_PAD], bf16, tag="xb")
            nc.sync.dma_start(xb[:], xb_in[:])
            xc = sb.tile([33, SPAT_PAD], bf16, tag="xc")
            nc.sync.dma_start(xc[:], xc_in[:])
            gb_sb = sb.tile([128, 4], f32, tag="gb")
            nc.sync.dma_start(gb_sb[:], gb_in[:])
            w_sb = sb.tile([128, N_KTILES * 2 * F], bf16, tag="w")
            for piece in range(4):
                a = piece * 10 * 2 * F
                b = min(N_KTILES, (piece + 1) * 10) * 2 * F
                nc.sync.dma_start(w_sb[:, a:b], w_in[:, a:b])

            # --- warmups: ACT function table + PE HAM clock gate ---
            scratch = sb.tile([128, 8], bf16, tag="scratch")
            nc.vector.memset(scratch[:], 0.25)
            nc.scalar.activation(scratch[:, 4:8], scratch[:, 0:4],
                                 mybir.ActivationFunctionType.Silu)
            pw = ps_w.tile([4, 4], f32, tag="pw")
            for _ in range(WARMUP_MMS):
                nc.tensor.matmul(pw[:], scratch[:, 0:4], scratch[:, 0:4],
                                 start=True, stop=True)

            # --- features (bf16): 4 relu tiles + 3 silu packs, by range ---
            feats = [sb.tile([128, SPAT_PAD], bf16, name=f"feat{t}",
                             tag=f"feat{t}") for t in range(4)]
            packs = [sb.tile([128, SPAT_PAD], bf16, name="pack0", tag="pack0"),
                     sb.tile([128, SPAT_PAD], bf16, name="pack1", tag="pack1"),
                     sb.tile([33, SPAT_PAD], bf16, name="pack2", tag="pack2")]

            for r in range(4):
                cs = slice(RANGES[r], RANGES[r + 1])
                for t in range(4):      # DVE: (x + (-g)) max 0, per-part g
                    nc.vector.tensor_scalar(feats[t][:, cs], xr[:, cs],
                                            gb_sb[:, t:t + 1], 0.0,
                                            mybir.AluOpType.add,
                                            mybir.AluOpType.max)
                nc.scalar.activation(packs[0][:, cs], xa[:, cs],
                                     mybir.ActivationFunctionType.Silu)
                nc.scalar.activation(packs[1][:, cs], xb[:, cs],
                                     mybir.ActivationFunctionType.Silu)
                nc.scalar.activation(packs[2][0:32, cs], xc[0:32, cs],
                                     mybir.ActivationFunctionType.Silu)
                # constant-1 bias row (x*0 + 1 from real values, no NaNs)
                nc.vector.tensor_scalar(packs[2][32:33, cs], xc[32:33, cs],
                                        0.0, 1.0, mybir.AluOpType.mult,
                                        mybir.AluOpType.add)

            # --- conv: 39 k-tiles, chunk pairs on PE column halves ---
            def rhs_for(kt, q):
                if kt < 36:
                    tap, t = divmod(kt, 4)
                    i, j = divmod(tap, KW)
                    base = (CHUNK_ROWS * q + i) * W
                    return (feats[t][:, base:base + CHUNK_ROWS * W]
                            .rearrange("p (r w) -> p r w", w=W)[:, :, j:j + WO])
                p = kt - 36
                k = 33 if p == 2 else 128
                base = CHUNK_ROWS * q * W
                return (packs[p][0:k, base:base + CHUNK_ROWS * W]
                        .rearrange("p (r w) -> p r w", w=W)[:, :, 0:WO])

            for cp in range(N_CHUNKS // 2):
                po = ps.tile([128, NFREE], f32, tag="po")
                for kt in range(N_KTILES):
                    k = 33 if kt == 38 else 128
                    col = kt * 2 * F
                    for half in range(2):
                        q = 2 * cp + half
                        nc.tensor.matmul(
                            po[F * half:F * (half + 1), :]
                            .rearrange("f (r w) -> f r w", w=WO),
                            w_sb[0:k, col + F * half:col + F * (half + 1)],
                            rhs_for(kt, q),
                            start=(kt == 0), stop=(kt == N_KTILES - 1),
                        )
                o_sb = ob.tile([128, NFREE], f32, tag="osb")
                nc.scalar.copy(o_sb[0:F, :], po[0:F, :])
                nc.vector.tensor_copy(o_sb[F:128, :], po[F:128, :])
                for half in range(2):
                    q = 2 * cp + half
                    nc.sync.dma_start(
                        y_out[:, NFREE * q:NFREE * (q + 1)],
                        o_sb[F * half:F * (half + 1), :])

    nc.compile()
    return nc


def _get_program():
    global _COMPILED
    if _COMPILED is None:
        _COMPILED = _build_program()
    return _COMPILED


def build_in_maps(x, control_points, w_spline, w_silu, bias):
    x = np.asarray(x, dtype=np.float32)
    w_host = _build_weights(np.asarray(control_points, dtype=np.float32),
                            np.asarray(w_spline, dtype=np.float32),
                            np.asarray(w_silu, dtype=np.float32),
                            np.asarray(bias, dtype=np.float32))
    grid = np.linspace(-1.0, 1.0, G + 1, dtype=np.float64)
    gb = np.zeros((128, 4), dtype=np.float32)
    for t in range(4):
        for p in range(128):
            gb[p, t] = -grid[4 * t + p // 32]

    bf = ml_dtypes.bfloat16
    in_maps = []
    for core in range(N_CORES):
        b, half = divmod(core, 2)
        r0 = half * ROWS_PER_CORE
        slab = np.zeros((C, SPAT_PAD), dtype=np.float32)
        slab[:, :SPAT] = x[b, r0:r0 + IN_ROWS].reshape(SPAT, C).T
        xr = np.tile(np.minimum(slab, 1.0), (4, 1))              # [128, spat]

        def shifted(shifts, rows):
            out = np.zeros((rows, SPAT_PAD), dtype=np.float32)
            for r, sh in enumerate(shifts):
                out[32 * r:32 * r + C, :SPAT_PAD - sh] = slab[:, sh:]
            return out

        xa = shifted(PACK_SHIFTS[0], 128)
        xb = shifted(PACK_SHIFTS[1], 128)
        xc = shifted(PACK_SHIFTS[2], 33)
        in_maps.append({"xr": xr.astype(bf), "xa": xa.astype(bf),
                        "xb": xb.astype(bf), "xc": xc.astype(bf),
                        "w": w_host, "gb": gb})
    return in_maps


def unshard_output(results):
    out = np.empty((B, HO, WO, F), dtype=np.float32)
    for core in range(N_CORES):
        b, half = divmod(core, 2)
        r0 = half * ROWS_PER_CORE
        y = results[core]["y"]                       # [F, 2048]
        out[b, r0:r0 + ROWS_PER_CORE] = (
            y.reshape(F, ROWS_PER_CORE, WO).transpose(1, 2, 0))
    return out


def kernel(x, control_points, w_spline, w_silu, bias):
    in_maps = build_in_maps(x, control_points, w_spline, w_silu, bias)
    nc = _get_program()
    res = run_bass_kernel_spmd(nc, in_maps, list(range(N_CORES)))
    return unshard_output(res.results)
